# revision 14
# baseline (speedup 1.0000x reference)
"""LoFTR-style coarse matching (dual-softmax conf matrix + mutual-NN extraction)
on 8 Trainium2 NeuronCores.

Sharding: batch n = core//4, L-rows l0 = (core%4)*1200. Each core computes a
[1200, 4800] slice of conf = softmax(sim, axis=1) * softmax(sim, axis=2).

sim = feat_c0 @ feat_c1^T / (256 * 0.1) has small magnitude (|sim| < ~5), so
softmax needs no max-subtraction:
    conf[l,j] = exp(2*sim[l,j] - ln(srow[l]) - ln(scol[j]))
with srow[l] = sum_j exp(sim[l,j]), scol[j] = sum_l exp(sim[l,j]).

Two SPMD launches (an on-device AllReduce costs ~70us of firmware latency,
two launch overheads are cheaper):
  A) per-core partial stats: srow (rows are fully local) and scol partial over
     the core's 1200 rows (ones-vector matmul on the exp tiles).
  B) host combines scol over the 4 cores of each batch, computes
     alpha[l] = -ln(srow[l]) - C0 (per-partition ACT bias) and
     beta[j] = (C0 - ln(scol[j]))/120, applied as a rank-1 K=120 matmul
     (ones[120,P]^T @ beta-broadcast). K=120 keeps the PE HAM activity monitor
     fed; a K=1 aug matmul would pin the PE at the cold 1.2 GHz clock.
     The device then emits conf tiles = Exp(psum + alpha) and their row max.

Features are fed as f16 in the SBUF layout [128, 2, cols] (K on partitions,
per-partition contiguous for wide DMA descriptors); accumulation is f32 in
PSUM. conf rel err vs the f32 reference ~2e-3.
"""

from contextlib import ExitStack

import numpy as np

N_BATCH = 2
L = 4800
S = 4800
C = 256
THR = 0.2
TEMP = 0.1
SPRIME = (1.0 / C) / TEMP  # sim = SPRIME * (a . b)

R = 1200          # L-rows per core
P = 120           # row (partition) tile
W = 512           # col tile (PSUM bank)
RT = R // P       # 10 row tiles
SPAD = 5120       # S padded to 10*512
CT = SPAD // W    # 10 col tiles
NPAD = SPAD - S   # 320 zero-padded columns
KT = 2            # contraction tiles of 128
NWARM = 40        # PE warm-up matmuls during the input load window


def _build_programs():
    import concourse.bass as bass
    import concourse.bacc as bacc
    import concourse.tile as tile
    from concourse import mybir

    f16 = mybir.dt.float16
    f32 = mybir.dt.float32
    Exp = mybir.ActivationFunctionType.Exp

    def warmup(nc, feat, psum, stats, pw):
        """~40 small matmuls to keep the PE HAM busy while inputs load.
        The result lands in a real (ignored) output so DCE keeps them."""
        wsrc = feat.tile([P, 128], f16)
        nc.vector.memset(wsrc[:], 1.0)
        wps = psum.tile([P, pw], f32, tag="ps")
        for w in range(NWARM):
            nc.tensor.matmul(
                wps[0:1, 0:128], wsrc[:, 0:1], wsrc[:],
                start=(w == 0), stop=(w == NWARM - 1),
            )
        wsb = stats.tile([1, 128], f32)
        nc.vector.tensor_copy(out=wsb[:], in_=wps[0:1, 0:128])
        return wsb

    # ---------------- launch A: stats ----------------
    nca = bacc.Bacc("TRN2", target_bir_lowering=False, debug=False, num_devices=8)
    a_aT = nca.dram_tensor("aT", [128, KT, R], f16, kind="ExternalInput").ap()
    a_bT = nca.dram_tensor("bT", [128, KT, SPAD], f16, kind="ExternalInput").ap()
    a_srow = nca.dram_tensor("srow_parts", [P, RT * 5], f32, kind="ExternalOutput").ap()
    a_scol = nca.dram_tensor("scol", [1, SPAD], f32, kind="ExternalOutput").ap()
    a_warm = nca.dram_tensor("warm", [1, 128], f32, kind="ExternalOutput").ap()

    with tile.TileContext(nca) as tc, ExitStack() as ctx:
        nc = tc.nc
        feat = ctx.enter_context(tc.tile_pool(name="feat", bufs=1))
        epool = ctx.enter_context(tc.tile_pool(name="e", bufs=4))
        stats = ctx.enter_context(tc.tile_pool(name="stats", bufs=1))
        psum = ctx.enter_context(tc.tile_pool(name="ps", bufs=2, space="PSUM"))
        pscol = ctx.enter_context(tc.tile_pool(name="pscol", bufs=2, space="PSUM"))

        wsb = warmup(nc, feat, psum, stats, 2 * W)

        aT = feat.tile([128, KT, R], f16)
        nc.sync.dma_start(out=aT[:], in_=a_aT)
        bT_c = []
        for jc in range(5):
            t = feat.tile([128, KT, 2 * W], f16, tag=f"bT{jc}")
            nc.sync.dma_start(
                out=t[:], in_=a_bT[:, :, jc * 2 * W:(jc + 1) * 2 * W]
            )
            bT_c.append(t)
        ones = feat.tile([P, 1], f16)
        nc.vector.memset(ones[:], 1.0)

        srow_parts = stats.tile([P, RT, 5], f32)
        scol_sb = stats.tile([1, SPAD], f32)

        for jc in range(5):  # col-tile pairs j = 2jc, 2jc+1
            scol_ps = pscol.tile([1, 2 * W], f32)
            for i in range(RT):
                ps = psum.tile([P, 2 * W], f32, tag="ps")
                for u in range(2):
                    for h in range(KT):
                        nc.tensor.matmul(
                            ps[:, u * W:(u + 1) * W],
                            aT[:, h, i * P:(i + 1) * P],
                            bT_c[jc][:, h, u * W:(u + 1) * W],
                            start=(h == 0),
                            stop=(h == KT - 1),
                        )
                e = epool.tile([P, 2 * W], f16)
                nc.scalar.activation(out=e[:], in_=ps[:], func=Exp, scale=0.5)
                # row sums on the otherwise-idle Vector engine
                nc.vector.tensor_reduce(
                    out=srow_parts[:, i, jc:jc + 1], in_=e[:],
                    axis=mybir.AxisListType.X, op=mybir.AluOpType.add,
                )
                for u in range(2):
                    nc.tensor.matmul(
                        scol_ps[:, u * W:(u + 1) * W],
                        ones[:],
                        e[:, u * W:(u + 1) * W],
                        start=(i == 0),
                        stop=(i == RT - 1),
                    )
            nc.vector.tensor_copy(
                out=scol_sb[:, 2 * jc * W:(2 * jc + 2) * W], in_=scol_ps[:]
            )
        nc.sync.dma_start(out=a_scol, in_=scol_sb[:])
        nc.sync.dma_start(out=a_srow, in_=srow_parts[:])
        nc.sync.dma_start(out=a_warm, in_=wsb[:])
    nca.compile()

    # ---------------- launch B: conf ----------------
    ncb = bacc.Bacc("TRN2", target_bir_lowering=False, debug=False, num_devices=8)
    b_aT = ncb.dram_tensor("aT", [128, KT, R], f16, kind="ExternalInput").ap()
    b_bT = ncb.dram_tensor("bT", [128, KT, SPAD], f16, kind="ExternalInput").ap()
    b_beta = ncb.dram_tensor("beta", [1, SPAD], f16, kind="ExternalInput").ap()
    b_alpha = ncb.dram_tensor("alpha", [P, RT], f32, kind="ExternalInput").ap()
    b_conf = ncb.dram_tensor("conf", [R, S], f32, kind="ExternalOutput").ap()
    b_rmax = ncb.dram_tensor("rowmax", [P, RT], f32, kind="ExternalOutput").ap()
    b_warm = ncb.dram_tensor("warm", [1, 128], f32, kind="ExternalOutput").ap()

    # col-tile batches per ACT instruction (PSUM pool: 2 bufs x 4 banks)
    CHUNKS = [(0, 4), (4, 8), (8, 10)]

    with tile.TileContext(ncb) as tc, ExitStack() as ctx:
        nc = tc.nc
        feat = ctx.enter_context(tc.tile_pool(name="feat", bufs=1))
        stage = ctx.enter_context(tc.tile_pool(name="stage", bufs=3))
        stats = ctx.enter_context(tc.tile_pool(name="stats", bufs=1))
        psum = ctx.enter_context(tc.tile_pool(name="ps", bufs=2, space="PSUM"))

        wsb = warmup(nc, feat, psum, stats, 4 * W)

        aT = feat.tile([128, KT, R], f16)
        nc.sync.dma_start(out=aT[:], in_=b_aT)
        bT_c, aug_c = [], []
        for ci, (j0, j1) in enumerate(CHUNKS):
            nw = j1 - j0
            t = feat.tile([128, KT, nw * W], f16, tag=f"bT{ci}")
            nc.sync.dma_start(out=t[:], in_=b_bT[:, :, j0 * W:j1 * W])
            bT_c.append(t)
            # beta/120 broadcast to the 120 partitions of the aug matmul rhs
            u = feat.tile([P, nw * W], f16, tag=f"aug{ci}")
            src = b_beta[0:1, j0 * W:j1 * W]
            nc.sync.dma_start(
                out=u[:],
                in_=bass.AP(tensor=src.tensor, offset=src.offset,
                            ap=[[0, P], src.ap[1]]),
            )
            aug_c.append(u)
        ones_aug = feat.tile([P, P], f16)
        nc.vector.memset(ones_aug[:], 1.0)
        alpha = stats.tile([P, RT], f32)
        nc.sync.dma_start(out=alpha[:], in_=b_alpha)
        rmax = stats.tile([P, RT], f32)

        for i in range(RT):
            st = stage.tile([P, SPAD], f32)
            maxp = stage.tile([P, len(CHUNKS)], f32, tag="maxp")
            for ci, (j0, j1) in enumerate(CHUNKS):
                nw = j1 - j0
                ps = psum.tile([P, 4 * W], f32, tag="ps")
                for u in range(nw):
                    for h in range(KT):
                        nc.tensor.matmul(
                            ps[:, u * W:(u + 1) * W],
                            aT[:, h, i * P:(i + 1) * P],
                            bT_c[ci][:, h, u * W:(u + 1) * W],
                            start=(h == 0),
                            stop=False,
                        )
                    nc.tensor.matmul(
                        ps[:, u * W:(u + 1) * W],
                        ones_aug[:],
                        aug_c[ci][:, u * W:(u + 1) * W],
                        start=False,
                        stop=True,
                    )
                nc.scalar.activation(
                    out=st[:, j0 * W:j1 * W], in_=ps[:, 0:nw * W], func=Exp,
                    scale=1.0, bias=alpha[:, i:i + 1],
                )
                # per-chunk: conf columns are valid up to S; DMA + running max
                c1 = min(j1 * W, S)
                nc.vector.tensor_reduce(
                    out=maxp[:, ci:ci + 1], in_=st[:, j0 * W:c1],
                    axis=mybir.AxisListType.X, op=mybir.AluOpType.max,
                )
                nc.sync.dma_start(
                    out=b_conf[i * P:(i + 1) * P, j0 * W:c1],
                    in_=st[:, j0 * W:c1],
                )
            nc.vector.tensor_reduce(
                out=rmax[:, i:i + 1], in_=maxp[:],
                axis=mybir.AxisListType.X, op=mybir.AluOpType.max,
            )
        nc.sync.dma_start(out=b_rmax, in_=rmax[:])
        nc.sync.dma_start(out=b_warm, in_=wsb[:])
    ncb.compile()

    return nca, ncb


_PROGRAMS = None


def _programs():
    global _PROGRAMS
    if _PROGRAMS is None:
        _PROGRAMS = _build_programs()
    return _PROGRAMS


def _run(nc, in_maps):
    from concourse.bass_utils import run_bass_kernel_spmd
    return run_bass_kernel_spmd(nc, in_maps, list(range(8))).results


def kernel(feat_c0, feat_c1, mask_c0, mask_c1):
    feat_c0 = np.asarray(feat_c0, dtype=np.float32)
    feat_c1 = np.asarray(feat_c1, dtype=np.float32)
    mask_c0 = np.asarray(mask_c0)
    mask_c1 = np.asarray(mask_c1)
    if not (mask_c0.all() and mask_c1.all()):
        return _kernel_numpy(feat_c0, feat_c1, mask_c0, mask_c1)

    nca, ncb = _programs()

    # ---- host prep: f16 features in SBUF layout [128, KT, cols]
    aT_np = []  # per core
    for c in range(8):
        n, l0 = c // 4, (c % 4) * R
        blk = (feat_c0[n, l0:l0 + R, :].T * (2.0 * SPRIME)).astype(np.float16)
        aT_np.append(np.ascontiguousarray(blk.reshape(KT, 128, R).transpose(1, 0, 2)))
    bT_np = []  # per batch
    for n in range(N_BATCH):
        blk = np.zeros((C, SPAD), np.float16)
        blk[:, 0:S] = feat_c1[n].T.astype(np.float16)
        bT_np.append(np.ascontiguousarray(blk.reshape(KT, 128, SPAD).transpose(1, 0, 2)))

    in_a = [{"aT": aT_np[c], "bT": bT_np[c // 4]} for c in range(8)]
    res_a = _run(nca, in_a)

    # ---- combine stats on host
    srow = np.empty((N_BATCH, L), np.float64)
    scol = np.zeros((N_BATCH, S), np.float64)
    for c in range(8):
        n, l0 = c // 4, (c % 4) * R
        parts = res_a[c]["srow_parts"].reshape(P, RT, 5).astype(np.float64)
        rs = parts.sum(axis=2)  # [P, RT]
        # the zero-padded columns contribute exp(0) = 1 each to the last chunk
        rs -= float(NPAD)
        srow[n, l0:l0 + R] = rs.T.reshape(R)
        scol[n] += res_a[c]["scol"][0, 0:S].astype(np.float64)

    ln_srow = np.log(srow)  # [N, L]
    ln_scol = np.log(scol)  # [N, S]
    c0 = ln_scol.mean(axis=1)  # per batch centering for f16 beta

    alpha_np = []
    for c in range(8):
        n, l0 = c // 4, (c % 4) * R
        al = (-ln_srow[n, l0:l0 + R] - c0[n]).astype(np.float32)
        alpha_np.append(np.ascontiguousarray(al.reshape(RT, P).T))  # [P, RT]
    beta_np = []
    for n in range(N_BATCH):
        row = np.zeros((1, SPAD), np.float16)
        row[0, 0:S] = ((c0[n] - ln_scol[n]) / P).astype(np.float16)
        beta_np.append(row)

    in_b = [
        {"aT": aT_np[c], "bT": bT_np[c // 4], "beta": beta_np[c // 4],
         "alpha": alpha_np[c]} for c in range(8)
    ]
    res_b = _run(ncb, in_b)

    # ---- assemble outputs
    conf = np.empty((N_BATCH, L, S), np.float32)
    maxs = np.empty((N_BATCH, L), np.float32)
    for c in range(8):
        n, l0 = c // 4, (c % 4) * R
        conf[n, l0:l0 + R, :] = res_b[c]["conf"]
        maxs[n, l0:l0 + R] = res_b[c]["rowmax"].T.reshape(R)

    mask_v = np.zeros((N_BATCH, L), bool)
    all_j_ids = np.zeros((N_BATCH, L), np.int32)
    mconf = np.zeros((N_BATCH, L), np.float32)
    cand = maxs > THR  # exact: device rowmax is the bitwise max of returned conf
    for n in range(N_BATCH):
        idx = np.nonzero(cand[n])[0]
        if idx.size == 0:
            continue
        colmax = conf[n].max(axis=0)  # [S]
        for l in idx:
            row = conf[n, l]
            m = (row > THR) & (row == row.max()) & (row == colmax)
            if m.any():
                j = int(np.argmax(m))
                mask_v[n, l] = True
                all_j_ids[n, l] = j
                mconf[n, l] = row[j]
    num_matches = mask_v.sum(axis=1).astype(np.int32)
    return conf, mask_v, all_j_ids, mconf, num_matches


def _kernel_numpy(feat_c0, feat_c1, mask_c0, mask_c1):
    """General-mask fallback (not used for the spec's all-ones masks)."""
    INF = 1e9
    conf = np.empty((N_BATCH, L, S), np.float32)
    for n in range(N_BATCH):
        sim = (feat_c0[n] / 16.0) @ (feat_c1[n] / 16.0).T / TEMP
        valid = mask_c0[n][:, None] & mask_c1[n][None, :]
        sim = np.where(valid, sim, -INF).astype(np.float32)
        e1 = np.exp(sim - sim.max(axis=0, keepdims=True))
        s1 = e1 / e1.sum(axis=0, keepdims=True)
        e2 = np.exp(sim - sim.max(axis=1, keepdims=True))
        s2 = e2 / e2.sum(axis=1, keepdims=True)
        conf[n] = s1 * s2
    max_s = conf.max(axis=2, keepdims=True)
    max_l = conf.max(axis=1, keepdims=True)
    mask = (conf > THR) & (conf == max_s) & (conf == max_l)
    mask_v = mask.any(axis=2)
    all_j_ids = np.argmax(mask, axis=2).astype(np.int32)
    mconf = np.take_along_axis(conf, all_j_ids[..., None], axis=2)[..., 0]
    mconf = mconf * mask_v.astype(np.float32)
    num_matches = mask_v.sum(axis=1).astype(np.int32)
    return conf, mask_v, all_j_ids, mconf, num_matches


# revision 19
# speedup vs baseline: 1.1550x; 1.1550x over previous
"""LoFTR-style coarse matching (dual-softmax conf matrix + mutual-NN extraction)
on 8 Trainium2 NeuronCores.

Sharding: batch n = core//4, L-rows l0 = (core%4)*1200. Each core computes a
[1200, 4800] slice of conf = softmax(sim, axis=1) * softmax(sim, axis=2).

sim = feat_c0 @ feat_c1^T / (256 * 0.1) has small magnitude (|sim| < ~5), so
softmax needs no max-subtraction:
    conf[l,j] = exp(2*sim[l,j] - ln(srow[l]) - ln(scol[j]))
with srow[l] = sum_j exp(sim[l,j]), scol[j] = sum_l exp(sim[l,j]).

Two SPMD launches (an on-device AllReduce costs ~70us of firmware latency,
two launch overheads are cheaper):
  A) per-core partial stats: srow (rows are fully local) and scol partial over
     the core's 1200 rows (ones-vector matmul on the exp tiles).
  B) host combines scol over the 4 cores of each batch, computes
     alpha[l] = -ln(srow[l]) - C0 (per-partition ACT bias) and
     beta[j] = (C0 - ln(scol[j]))/120, applied as a rank-1 K=120 matmul
     (ones[120,P]^T @ beta-broadcast). K=120 keeps the PE HAM activity monitor
     fed; a K=1 aug matmul would pin the PE at the cold 1.2 GHz clock.
     The device then emits conf tiles = Exp(psum + alpha) and their row max.

Features are fed as f16 in the SBUF layout [128, 2, cols] (K on partitions,
per-partition contiguous for wide DMA descriptors); accumulation is f32 in
PSUM. conf rel err vs the f32 reference ~2e-3.
"""

from contextlib import ExitStack

import numpy as np

N_BATCH = 2
L = 4800
S = 4800
C = 256
THR = 0.2
TEMP = 0.1
SPRIME = (1.0 / C) / TEMP  # sim = SPRIME * (a . b)

R = 1200          # L-rows per core
P = 120           # row (partition) tile
W = 512           # col tile (PSUM bank)
RT = R // P       # 10 row tiles
SPAD = 5120       # S padded to 10*512
CT = SPAD // W    # 10 col tiles
NPAD = SPAD - S   # 320 zero-padded columns
KT = 2            # contraction tiles of 128
NWARM = 52        # PE warm-up matmuls during the input load window
WARMN = 160       # free dim of each warm-up matmul


def _build_programs():
    import concourse.bass as bass
    import concourse.bacc as bacc
    import concourse.tile as tile
    from concourse import mybir

    f16 = mybir.dt.float16
    f32 = mybir.dt.float32
    Exp = mybir.ActivationFunctionType.Exp

    def warmup(nc, feat, psum, stats, pw):
        """Small matmuls to keep the PE HAM busy while inputs load.
        The result lands in a real (ignored) output so DCE keeps them."""
        wsrc = feat.tile([P, WARMN], f16)
        nc.vector.memset(wsrc[:], 1.0)
        wps = psum.tile([P, pw], f32, tag="ps")
        for w in range(NWARM):
            nc.tensor.matmul(
                wps[0:1, 0:WARMN], wsrc[:, 0:1], wsrc[:],
                start=(w == 0), stop=(w == NWARM - 1),
            )
        wsb = stats.tile([1, 128], f32)
        nc.vector.tensor_copy(out=wsb[:], in_=wps[0:1, 0:128])
        return wsb

    # ---------------- launch A: stats ----------------
    nca = bacc.Bacc("TRN2", target_bir_lowering=False, debug=False, num_devices=8)
    a_aT = nca.dram_tensor("aT", [128, KT, R], f16, kind="ExternalInput").ap()
    a_bT = nca.dram_tensor("bT", [128, KT, SPAD], f16, kind="ExternalInput").ap()
    a_srow = nca.dram_tensor("srow_parts", [P, RT * 5], f32, kind="ExternalOutput").ap()
    a_scol = nca.dram_tensor("scol", [1, SPAD], f32, kind="ExternalOutput").ap()
    a_warm = nca.dram_tensor("warm", [1, 128], f32, kind="ExternalOutput").ap()

    with tile.TileContext(nca) as tc, ExitStack() as ctx:
        nc = tc.nc
        feat = ctx.enter_context(tc.tile_pool(name="feat", bufs=1))
        epool = ctx.enter_context(tc.tile_pool(name="e", bufs=4))
        stats = ctx.enter_context(tc.tile_pool(name="stats", bufs=1))
        psum = ctx.enter_context(tc.tile_pool(name="ps", bufs=2, space="PSUM"))
        pscol = ctx.enter_context(tc.tile_pool(name="pscol", bufs=2, space="PSUM"))

        wsb = warmup(nc, feat, psum, stats, 2 * W)

        aT = feat.tile([128, KT, R], f16)
        nc.sync.dma_start(out=aT[:, :, 0:P], in_=a_aT[:, :, 0:P])
        nc.sync.dma_start(out=aT[:, :, P:R], in_=a_aT[:, :, P:R])
        bT_c = []
        for jc in range(5):
            t = feat.tile([128, KT, 2 * W], f16, tag=f"bT{jc}")
            if jc == 0:
                nc.sync.dma_start(out=t[:, :, 0:W], in_=a_bT[:, :, 0:W])
                nc.sync.dma_start(out=t[:, :, W:2 * W], in_=a_bT[:, :, W:2 * W])
            else:
                nc.sync.dma_start(
                    out=t[:], in_=a_bT[:, :, jc * 2 * W:(jc + 1) * 2 * W]
                )
            bT_c.append(t)
        ones = feat.tile([P, 1], f16)
        nc.vector.memset(ones[:], 1.0)

        srow_parts = stats.tile([P, RT, 5], f32)
        scol_sb = stats.tile([1, SPAD], f32)

        for jc in range(5):  # col-tile pairs j = 2jc, 2jc+1
            scol_ps = pscol.tile([1, 2 * W], f32)
            for i in range(RT):
                ps = psum.tile([P, 2 * W], f32, tag="ps")
                for u in range(2):
                    for h in range(KT):
                        nc.tensor.matmul(
                            ps[:, u * W:(u + 1) * W],
                            aT[:, h, i * P:(i + 1) * P],
                            bT_c[jc][:, h, u * W:(u + 1) * W],
                            start=(h == 0),
                            stop=(h == KT - 1),
                        )
                e = epool.tile([P, 2 * W], f16)
                nc.scalar.activation(out=e[:], in_=ps[:], func=Exp, scale=0.5)
                # row sums on the otherwise-idle Vector engine
                nc.vector.tensor_reduce(
                    out=srow_parts[:, i, jc:jc + 1], in_=e[:],
                    axis=mybir.AxisListType.X, op=mybir.AluOpType.add,
                )
                for u in range(2):
                    nc.tensor.matmul(
                        scol_ps[:, u * W:(u + 1) * W],
                        ones[:],
                        e[:, u * W:(u + 1) * W],
                        start=(i == 0),
                        stop=(i == RT - 1),
                    )
            nc.vector.tensor_copy(
                out=scol_sb[:, 2 * jc * W:(2 * jc + 2) * W], in_=scol_ps[:]
            )
            # stream stats out as each column-pair finishes
            nc.sync.dma_start(
                out=a_scol[:, 2 * jc * W:(2 * jc + 2) * W],
                in_=scol_sb[:, 2 * jc * W:(2 * jc + 2) * W],
            )
            nc.sync.dma_start(
                out=a_srow.rearrange("p (i j) -> p i j", j=5)[:, :, jc:jc + 1],
                in_=srow_parts[:, :, jc:jc + 1],
            )
        nc.sync.dma_start(out=a_warm, in_=wsb[:])
    nca.compile()

    # ---------------- launch B: conf ----------------
    ncb = bacc.Bacc("TRN2", target_bir_lowering=False, debug=False, num_devices=8)
    b_aT = ncb.dram_tensor("aT", [128, KT, R], f16, kind="ExternalInput").ap()
    b_bT = ncb.dram_tensor("bT", [128, KT, SPAD], f16, kind="ExternalInput").ap()
    b_beta = ncb.dram_tensor("beta", [1, SPAD], f16, kind="ExternalInput").ap()
    b_alpha = ncb.dram_tensor("alpha", [P, RT], f32, kind="ExternalInput").ap()
    b_conf = ncb.dram_tensor("conf", [R, S], f32, kind="ExternalOutput").ap()
    b_rmax = ncb.dram_tensor("rowmax", [P, RT], f32, kind="ExternalOutput").ap()
    b_warm = ncb.dram_tensor("warm", [1, 128], f32, kind="ExternalOutput").ap()

    # col-tile batches per ACT instruction (PSUM pool: 2 bufs x 4 banks)
    CHUNKS = [(0, 4), (4, 8), (8, 10)]

    with tile.TileContext(ncb) as tc, ExitStack() as ctx:
        nc = tc.nc
        feat = ctx.enter_context(tc.tile_pool(name="feat", bufs=1))
        stage = ctx.enter_context(tc.tile_pool(name="stage", bufs=3))
        stats = ctx.enter_context(tc.tile_pool(name="stats", bufs=1))
        psum = ctx.enter_context(tc.tile_pool(name="ps", bufs=2, space="PSUM"))

        wsb = warmup(nc, feat, psum, stats, 4 * W)

        # small inputs on the gpsimd DMA path so they don't queue behind the
        # feature loads
        alpha = stats.tile([P, RT], f32)
        nc.gpsimd.dma_start(out=alpha[:], in_=b_alpha)
        aT = feat.tile([128, KT, R], f16)
        nc.sync.dma_start(out=aT[:, :, 0:P], in_=b_aT[:, :, 0:P])
        nc.sync.dma_start(out=aT[:, :, P:R], in_=b_aT[:, :, P:R])
        bT_c, aug_c = [], []
        for ci, (j0, j1) in enumerate(CHUNKS):
            nw = j1 - j0
            t = feat.tile([128, KT, nw * W], f16, tag=f"bT{ci}")
            if ci == 0:
                nc.sync.dma_start(out=t[:, :, 0:W], in_=b_bT[:, :, 0:W])
                nc.sync.dma_start(
                    out=t[:, :, W:nw * W], in_=b_bT[:, :, W:nw * W]
                )
            else:
                nc.sync.dma_start(out=t[:], in_=b_bT[:, :, j0 * W:j1 * W])
            bT_c.append(t)
            # beta/120 broadcast to the 120 partitions of the aug matmul rhs
            u = feat.tile([P, nw * W], f16, tag=f"aug{ci}")
            src = b_beta[0:1, j0 * W:j1 * W]
            nc.gpsimd.dma_start(
                out=u[:],
                in_=bass.AP(tensor=src.tensor, offset=src.offset,
                            ap=[[0, P], src.ap[1]]),
            )
            aug_c.append(u)
        ones_aug = feat.tile([P, P], f16)
        nc.vector.memset(ones_aug[:], 1.0)
        rmax = stats.tile([P, RT], f32)

        for i in range(RT):
            st = stage.tile([P, SPAD], f32)
            maxp = stage.tile([P, len(CHUNKS)], f32, tag="maxp")
            for ci, (j0, j1) in enumerate(CHUNKS):
                nw = j1 - j0
                ps = psum.tile([P, 4 * W], f32, tag="ps")
                for u in range(nw):
                    for h in range(KT):
                        nc.tensor.matmul(
                            ps[:, u * W:(u + 1) * W],
                            aT[:, h, i * P:(i + 1) * P],
                            bT_c[ci][:, h, u * W:(u + 1) * W],
                            start=(h == 0),
                            stop=False,
                        )
                    nc.tensor.matmul(
                        ps[:, u * W:(u + 1) * W],
                        ones_aug[:],
                        aug_c[ci][:, u * W:(u + 1) * W],
                        start=False,
                        stop=True,
                    )
                nc.scalar.activation(
                    out=st[:, j0 * W:j1 * W], in_=ps[:, 0:nw * W], func=Exp,
                    scale=1.0, bias=alpha[:, i:i + 1],
                )
                # per-chunk: conf columns are valid up to S; DMA + running max
                c1 = min(j1 * W, S)
                nc.vector.tensor_reduce(
                    out=maxp[:, ci:ci + 1], in_=st[:, j0 * W:c1],
                    axis=mybir.AxisListType.X, op=mybir.AluOpType.max,
                )
                nc.sync.dma_start(
                    out=b_conf[i * P:(i + 1) * P, j0 * W:c1],
                    in_=st[:, j0 * W:c1],
                )
            nc.vector.tensor_reduce(
                out=rmax[:, i:i + 1], in_=maxp[:],
                axis=mybir.AxisListType.X, op=mybir.AluOpType.max,
            )
        nc.sync.dma_start(out=b_rmax, in_=rmax[:])
        nc.sync.dma_start(out=b_warm, in_=wsb[:])
    ncb.compile()

    return nca, ncb


_PROGRAMS = None


def _programs():
    global _PROGRAMS
    if _PROGRAMS is None:
        _PROGRAMS = _build_programs()
    return _PROGRAMS


def _run(nc, in_maps):
    from concourse.bass_utils import run_bass_kernel_spmd
    return run_bass_kernel_spmd(nc, in_maps, list(range(8))).results


def kernel(feat_c0, feat_c1, mask_c0, mask_c1):
    feat_c0 = np.asarray(feat_c0, dtype=np.float32)
    feat_c1 = np.asarray(feat_c1, dtype=np.float32)
    mask_c0 = np.asarray(mask_c0)
    mask_c1 = np.asarray(mask_c1)
    if not (mask_c0.all() and mask_c1.all()):
        return _kernel_numpy(feat_c0, feat_c1, mask_c0, mask_c1)

    nca, ncb = _programs()

    # ---- host prep: f16 features in SBUF layout [128, KT, cols]
    aT_np = []  # per core
    for c in range(8):
        n, l0 = c // 4, (c % 4) * R
        blk = (feat_c0[n, l0:l0 + R, :].T * (2.0 * SPRIME)).astype(np.float16)
        aT_np.append(np.ascontiguousarray(blk.reshape(KT, 128, R).transpose(1, 0, 2)))
    bT_np = []  # per batch
    for n in range(N_BATCH):
        blk = np.zeros((C, SPAD), np.float16)
        blk[:, 0:S] = feat_c1[n].T.astype(np.float16)
        bT_np.append(np.ascontiguousarray(blk.reshape(KT, 128, SPAD).transpose(1, 0, 2)))

    in_a = [{"aT": aT_np[c], "bT": bT_np[c // 4]} for c in range(8)]
    res_a = _run(nca, in_a)

    # ---- combine stats on host
    srow = np.empty((N_BATCH, L), np.float64)
    scol = np.zeros((N_BATCH, S), np.float64)
    for c in range(8):
        n, l0 = c // 4, (c % 4) * R
        parts = res_a[c]["srow_parts"].reshape(P, RT, 5).astype(np.float64)
        rs = parts.sum(axis=2)  # [P, RT]
        # the zero-padded columns contribute exp(0) = 1 each to the last chunk
        rs -= float(NPAD)
        srow[n, l0:l0 + R] = rs.T.reshape(R)
        scol[n] += res_a[c]["scol"][0, 0:S].astype(np.float64)

    ln_srow = np.log(srow)  # [N, L]
    ln_scol = np.log(scol)  # [N, S]
    c0 = ln_scol.mean(axis=1)  # per batch centering for f16 beta

    alpha_np = []
    for c in range(8):
        n, l0 = c // 4, (c % 4) * R
        al = (-ln_srow[n, l0:l0 + R] - c0[n]).astype(np.float32)
        alpha_np.append(np.ascontiguousarray(al.reshape(RT, P).T))  # [P, RT]
    beta_np = []
    for n in range(N_BATCH):
        row = np.zeros((1, SPAD), np.float16)
        row[0, 0:S] = ((c0[n] - ln_scol[n]) / P).astype(np.float16)
        beta_np.append(row)

    in_b = [
        {"aT": aT_np[c], "bT": bT_np[c // 4], "beta": beta_np[c // 4],
         "alpha": alpha_np[c]} for c in range(8)
    ]
    res_b = _run(ncb, in_b)

    # ---- assemble outputs
    conf = np.empty((N_BATCH, L, S), np.float32)
    maxs = np.empty((N_BATCH, L), np.float32)
    for c in range(8):
        n, l0 = c // 4, (c % 4) * R
        conf[n, l0:l0 + R, :] = res_b[c]["conf"]
        maxs[n, l0:l0 + R] = res_b[c]["rowmax"].T.reshape(R)

    mask_v = np.zeros((N_BATCH, L), bool)
    all_j_ids = np.zeros((N_BATCH, L), np.int32)
    mconf = np.zeros((N_BATCH, L), np.float32)
    cand = maxs > THR  # exact: device rowmax is the bitwise max of returned conf
    for n in range(N_BATCH):
        idx = np.nonzero(cand[n])[0]
        if idx.size == 0:
            continue
        colmax = conf[n].max(axis=0)  # [S]
        for l in idx:
            row = conf[n, l]
            m = (row > THR) & (row == row.max()) & (row == colmax)
            if m.any():
                j = int(np.argmax(m))
                mask_v[n, l] = True
                all_j_ids[n, l] = j
                mconf[n, l] = row[j]
    num_matches = mask_v.sum(axis=1).astype(np.int32)
    return conf, mask_v, all_j_ids, mconf, num_matches


def _kernel_numpy(feat_c0, feat_c1, mask_c0, mask_c1):
    """General-mask fallback (not used for the spec's all-ones masks)."""
    INF = 1e9
    conf = np.empty((N_BATCH, L, S), np.float32)
    for n in range(N_BATCH):
        sim = (feat_c0[n] / 16.0) @ (feat_c1[n] / 16.0).T / TEMP
        valid = mask_c0[n][:, None] & mask_c1[n][None, :]
        sim = np.where(valid, sim, -INF).astype(np.float32)
        e1 = np.exp(sim - sim.max(axis=0, keepdims=True))
        s1 = e1 / e1.sum(axis=0, keepdims=True)
        e2 = np.exp(sim - sim.max(axis=1, keepdims=True))
        s2 = e2 / e2.sum(axis=1, keepdims=True)
        conf[n] = s1 * s2
    max_s = conf.max(axis=2, keepdims=True)
    max_l = conf.max(axis=1, keepdims=True)
    mask = (conf > THR) & (conf == max_s) & (conf == max_l)
    mask_v = mask.any(axis=2)
    all_j_ids = np.argmax(mask, axis=2).astype(np.int32)
    mconf = np.take_along_axis(conf, all_j_ids[..., None], axis=2)[..., 0]
    mconf = mconf * mask_v.astype(np.float32)
    num_matches = mask_v.sum(axis=1).astype(np.int32)
    return conf, mask_v, all_j_ids, mconf, num_matches


# revision 21
# speedup vs baseline: 1.1643x; 1.0081x over previous
"""LoFTR-style coarse matching (dual-softmax conf matrix + mutual-NN extraction)
on 8 Trainium2 NeuronCores.

Sharding: batch n = core//4, L-rows l0 = (core%4)*1200. Each core computes a
[1200, 4800] slice of conf = softmax(sim, axis=1) * softmax(sim, axis=2).

sim = feat_c0 @ feat_c1^T / (256 * 0.1) has small magnitude (|sim| < ~5), so
softmax needs no max-subtraction:
    conf[l,j] = exp(2*sim[l,j] - ln(srow[l]) - ln(scol[j]))
with srow[l] = sum_j exp(sim[l,j]), scol[j] = sum_l exp(sim[l,j]).

Two SPMD launches (an on-device AllReduce costs ~70us of firmware latency,
two launch overheads are cheaper):
  A) per-core partial stats: srow (rows are fully local) and scol partial over
     the core's 1200 rows (ones-vector matmul on the exp tiles).
  B) host combines scol over the 4 cores of each batch, computes
     alpha[l] = -ln(srow[l]) - C0 (per-partition ACT bias) and
     beta[j] = (C0 - ln(scol[j]))/120, applied as a rank-1 K=120 matmul
     (ones[120,P]^T @ beta-broadcast). K=120 keeps the PE HAM activity monitor
     fed; a K=1 aug matmul would pin the PE at the cold 1.2 GHz clock.
     The device then emits conf tiles = Exp(psum + alpha) and their row max.

Features are fed as f16 in the SBUF layout [128, 2, cols] (K on partitions,
per-partition contiguous for wide DMA descriptors); accumulation is f32 in
PSUM. conf rel err vs the f32 reference ~2e-3.
"""

from contextlib import ExitStack

import numpy as np

N_BATCH = 2
L = 4800
S = 4800
C = 256
THR = 0.2
TEMP = 0.1
SPRIME = (1.0 / C) / TEMP  # sim = SPRIME * (a . b)

R = 1200          # L-rows per core
P = 120           # row (partition) tile
W = 512           # col tile (PSUM bank)
RT = R // P       # 10 row tiles
SPAD = 5120       # S padded to 10*512
CT = SPAD // W    # 10 col tiles
NPAD = SPAD - S   # 320 zero-padded columns
KT = 2            # contraction tiles of 128
NWARM = 52        # PE warm-up matmuls during the input load window
WARMN = 160       # free dim of each warm-up matmul


def _build_programs():
    import concourse.bass as bass
    import concourse.bacc as bacc
    import concourse.tile as tile
    from concourse import mybir

    f16 = mybir.dt.float16
    f32 = mybir.dt.float32
    Exp = mybir.ActivationFunctionType.Exp

    def warmup(nc, feat, psum, stats, pw):
        """Small matmuls to keep the PE HAM busy while inputs load.
        The result lands in a real (ignored) output so DCE keeps them."""
        wsrc = feat.tile([P, WARMN], f16)
        nc.vector.memset(wsrc[:], 1.0)
        wps = psum.tile([P, pw], f32, tag="ps")
        for w in range(NWARM):
            nc.tensor.matmul(
                wps[0:1, 0:WARMN], wsrc[:, 0:1], wsrc[:],
                start=(w == 0), stop=(w == NWARM - 1),
            )
        wsb = stats.tile([1, 128], f32)
        nc.vector.tensor_copy(out=wsb[:], in_=wps[0:1, 0:128])
        return wsb

    # ---------------- launch A: stats ----------------
    nca = bacc.Bacc("TRN2", target_bir_lowering=False, debug=False, num_devices=8)
    a_aT = nca.dram_tensor("aT", [128, KT, R], f16, kind="ExternalInput").ap()
    a_bT = nca.dram_tensor("bT", [128, KT, SPAD], f16, kind="ExternalInput").ap()
    a_srow = nca.dram_tensor("srow_parts", [P, RT * 5], f32, kind="ExternalOutput").ap()
    a_scol = nca.dram_tensor("scol", [1, SPAD], f32, kind="ExternalOutput").ap()
    a_warm = nca.dram_tensor("warm", [1, 128], f32, kind="ExternalOutput").ap()

    with tile.TileContext(nca) as tc, ExitStack() as ctx:
        nc = tc.nc
        feat = ctx.enter_context(tc.tile_pool(name="feat", bufs=1))
        epool = ctx.enter_context(tc.tile_pool(name="e", bufs=6))
        stats = ctx.enter_context(tc.tile_pool(name="stats", bufs=1))
        psum = ctx.enter_context(tc.tile_pool(name="ps", bufs=2, space="PSUM"))
        pscol = ctx.enter_context(tc.tile_pool(name="pscol", bufs=2, space="PSUM"))

        wsb = warmup(nc, feat, psum, stats, 2 * W)

        aT = feat.tile([128, KT, R], f16)
        nc.sync.dma_start(out=aT[:, :, 0:P], in_=a_aT[:, :, 0:P])
        nc.sync.dma_start(out=aT[:, :, P:R], in_=a_aT[:, :, P:R])
        bT_c = []
        for jc in range(5):
            t = feat.tile([128, KT, 2 * W], f16, tag=f"bT{jc}")
            if jc == 0:
                nc.sync.dma_start(out=t[:, :, 0:W], in_=a_bT[:, :, 0:W])
                nc.sync.dma_start(out=t[:, :, W:2 * W], in_=a_bT[:, :, W:2 * W])
            else:
                nc.sync.dma_start(
                    out=t[:], in_=a_bT[:, :, jc * 2 * W:(jc + 1) * 2 * W]
                )
            bT_c.append(t)
        ones = feat.tile([P, 1], f16)
        nc.vector.memset(ones[:], 1.0)

        srow_parts = stats.tile([P, RT, 5], f32)
        scol_sb = stats.tile([1, SPAD], f32)

        for jc in range(5):  # col-tile pairs j = 2jc, 2jc+1
            scol_ps = pscol.tile([1, 2 * W], f32)
            for i in range(RT):
                ps = psum.tile([P, 2 * W], f32, tag="ps")
                for u in range(2):
                    for h in range(KT):
                        nc.tensor.matmul(
                            ps[:, u * W:(u + 1) * W],
                            aT[:, h, i * P:(i + 1) * P],
                            bT_c[jc][:, h, u * W:(u + 1) * W],
                            start=(h == 0),
                            stop=(h == KT - 1),
                        )
                e = epool.tile([P, 2 * W], f16)
                nc.scalar.activation(out=e[:], in_=ps[:], func=Exp, scale=0.5)
                # row sums on the otherwise-idle Vector engine
                nc.vector.tensor_reduce(
                    out=srow_parts[:, i, jc:jc + 1], in_=e[:],
                    axis=mybir.AxisListType.X, op=mybir.AluOpType.add,
                )
                for u in range(2):
                    nc.tensor.matmul(
                        scol_ps[:, u * W:(u + 1) * W],
                        ones[:],
                        e[:, u * W:(u + 1) * W],
                        start=(i == 0),
                        stop=(i == RT - 1),
                    )
            nc.vector.tensor_copy(
                out=scol_sb[:, 2 * jc * W:(2 * jc + 2) * W], in_=scol_ps[:]
            )
            # stream stats out as each column-pair finishes
            nc.sync.dma_start(
                out=a_scol[:, 2 * jc * W:(2 * jc + 2) * W],
                in_=scol_sb[:, 2 * jc * W:(2 * jc + 2) * W],
            )
            nc.sync.dma_start(
                out=a_srow.rearrange("p (i j) -> p i j", j=5)[:, :, jc:jc + 1],
                in_=srow_parts[:, :, jc:jc + 1],
            )
        nc.sync.dma_start(out=a_warm, in_=wsb[:])
    nca.compile()

    # ---------------- launch B: conf ----------------
    ncb = bacc.Bacc("TRN2", target_bir_lowering=False, debug=False, num_devices=8)
    b_aT = ncb.dram_tensor("aT", [128, KT, R], f16, kind="ExternalInput").ap()
    b_bT = ncb.dram_tensor("bT", [128, KT, SPAD], f16, kind="ExternalInput").ap()
    b_beta = ncb.dram_tensor("beta", [1, SPAD], f16, kind="ExternalInput").ap()
    b_alpha = ncb.dram_tensor("alpha", [P, RT], f32, kind="ExternalInput").ap()
    b_conf = ncb.dram_tensor("conf", [R, S], f32, kind="ExternalOutput").ap()
    b_rmax = ncb.dram_tensor("rowmax", [P, RT], f32, kind="ExternalOutput").ap()
    b_warm = ncb.dram_tensor("warm", [1, 128], f32, kind="ExternalOutput").ap()

    # col-tile batches per ACT instruction (PSUM pool: 2 bufs x 4 banks)
    CHUNKS = [(0, 4), (4, 8), (8, 10)]

    with tile.TileContext(ncb) as tc, ExitStack() as ctx:
        nc = tc.nc
        feat = ctx.enter_context(tc.tile_pool(name="feat", bufs=1))
        stage = ctx.enter_context(tc.tile_pool(name="stage", bufs=4))
        stats = ctx.enter_context(tc.tile_pool(name="stats", bufs=1))
        psum = ctx.enter_context(tc.tile_pool(name="ps", bufs=2, space="PSUM"))

        wsb = warmup(nc, feat, psum, stats, 4 * W)

        # small inputs on the gpsimd DMA path so they don't queue behind the
        # feature loads
        alpha = stats.tile([P, RT], f32)
        nc.gpsimd.dma_start(out=alpha[:], in_=b_alpha)
        aT = feat.tile([128, KT, R], f16)
        nc.sync.dma_start(out=aT[:, :, 0:P], in_=b_aT[:, :, 0:P])
        nc.sync.dma_start(out=aT[:, :, P:R], in_=b_aT[:, :, P:R])
        bT_c, aug_c = [], []
        for ci, (j0, j1) in enumerate(CHUNKS):
            nw = j1 - j0
            t = feat.tile([128, KT, nw * W], f16, tag=f"bT{ci}")
            if ci == 0:
                nc.sync.dma_start(out=t[:, :, 0:W], in_=b_bT[:, :, 0:W])
                nc.sync.dma_start(
                    out=t[:, :, W:nw * W], in_=b_bT[:, :, W:nw * W]
                )
            else:
                nc.sync.dma_start(out=t[:], in_=b_bT[:, :, j0 * W:j1 * W])
            bT_c.append(t)
            # beta/120 broadcast to the 120 partitions of the aug matmul rhs
            u = feat.tile([P, nw * W], f16, tag=f"aug{ci}")
            src = b_beta[0:1, j0 * W:j1 * W]
            nc.gpsimd.dma_start(
                out=u[:],
                in_=bass.AP(tensor=src.tensor, offset=src.offset,
                            ap=[[0, P], src.ap[1]]),
            )
            aug_c.append(u)
        ones_aug = feat.tile([P, P], f16)
        nc.vector.memset(ones_aug[:], 1.0)
        rmax = stats.tile([P, RT], f32)

        for i in range(RT):
            st = stage.tile([P, SPAD], f32)
            maxp = stage.tile([P, len(CHUNKS)], f32, tag="maxp")
            for ci, (j0, j1) in enumerate(CHUNKS):
                nw = j1 - j0
                ps = psum.tile([P, 4 * W], f32, tag="ps")
                for u in range(nw):
                    for h in range(KT):
                        nc.tensor.matmul(
                            ps[:, u * W:(u + 1) * W],
                            aT[:, h, i * P:(i + 1) * P],
                            bT_c[ci][:, h, u * W:(u + 1) * W],
                            start=(h == 0),
                            stop=False,
                        )
                    nc.tensor.matmul(
                        ps[:, u * W:(u + 1) * W],
                        ones_aug[:],
                        aug_c[ci][:, u * W:(u + 1) * W],
                        start=False,
                        stop=True,
                    )
                nc.scalar.activation(
                    out=st[:, j0 * W:j1 * W], in_=ps[:, 0:nw * W], func=Exp,
                    scale=1.0, bias=alpha[:, i:i + 1],
                )
                # per-chunk: conf columns are valid up to S; DMA + running max
                c1 = min(j1 * W, S)
                nc.vector.tensor_reduce(
                    out=maxp[:, ci:ci + 1], in_=st[:, j0 * W:c1],
                    axis=mybir.AxisListType.X, op=mybir.AluOpType.max,
                )
                nc.sync.dma_start(
                    out=b_conf[i * P:(i + 1) * P, j0 * W:c1],
                    in_=st[:, j0 * W:c1],
                )
            nc.vector.tensor_reduce(
                out=rmax[:, i:i + 1], in_=maxp[:],
                axis=mybir.AxisListType.X, op=mybir.AluOpType.max,
            )
        nc.sync.dma_start(out=b_rmax, in_=rmax[:])
        nc.sync.dma_start(out=b_warm, in_=wsb[:])
    ncb.compile()

    return nca, ncb


_PROGRAMS = None


def _programs():
    global _PROGRAMS
    if _PROGRAMS is None:
        _PROGRAMS = _build_programs()
    return _PROGRAMS


def _run(nc, in_maps):
    from concourse.bass_utils import run_bass_kernel_spmd
    return run_bass_kernel_spmd(nc, in_maps, list(range(8))).results


def kernel(feat_c0, feat_c1, mask_c0, mask_c1):
    feat_c0 = np.asarray(feat_c0, dtype=np.float32)
    feat_c1 = np.asarray(feat_c1, dtype=np.float32)
    mask_c0 = np.asarray(mask_c0)
    mask_c1 = np.asarray(mask_c1)
    if not (mask_c0.all() and mask_c1.all()):
        return _kernel_numpy(feat_c0, feat_c1, mask_c0, mask_c1)

    nca, ncb = _programs()

    # ---- host prep: f16 features in SBUF layout [128, KT, cols]
    aT_np = []  # per core
    for c in range(8):
        n, l0 = c // 4, (c % 4) * R
        blk = (feat_c0[n, l0:l0 + R, :].T * (2.0 * SPRIME)).astype(np.float16)
        aT_np.append(np.ascontiguousarray(blk.reshape(KT, 128, R).transpose(1, 0, 2)))
    bT_np = []  # per batch
    for n in range(N_BATCH):
        blk = np.zeros((C, SPAD), np.float16)
        blk[:, 0:S] = feat_c1[n].T.astype(np.float16)
        bT_np.append(np.ascontiguousarray(blk.reshape(KT, 128, SPAD).transpose(1, 0, 2)))

    in_a = [{"aT": aT_np[c], "bT": bT_np[c // 4]} for c in range(8)]
    res_a = _run(nca, in_a)

    # ---- combine stats on host
    srow = np.empty((N_BATCH, L), np.float64)
    scol = np.zeros((N_BATCH, S), np.float64)
    for c in range(8):
        n, l0 = c // 4, (c % 4) * R
        parts = res_a[c]["srow_parts"].reshape(P, RT, 5).astype(np.float64)
        rs = parts.sum(axis=2)  # [P, RT]
        # the zero-padded columns contribute exp(0) = 1 each to the last chunk
        rs -= float(NPAD)
        srow[n, l0:l0 + R] = rs.T.reshape(R)
        scol[n] += res_a[c]["scol"][0, 0:S].astype(np.float64)

    ln_srow = np.log(srow)  # [N, L]
    ln_scol = np.log(scol)  # [N, S]
    c0 = ln_scol.mean(axis=1)  # per batch centering for f16 beta

    alpha_np = []
    for c in range(8):
        n, l0 = c // 4, (c % 4) * R
        al = (-ln_srow[n, l0:l0 + R] - c0[n]).astype(np.float32)
        alpha_np.append(np.ascontiguousarray(al.reshape(RT, P).T))  # [P, RT]
    beta_np = []
    for n in range(N_BATCH):
        row = np.zeros((1, SPAD), np.float16)
        row[0, 0:S] = ((c0[n] - ln_scol[n]) / P).astype(np.float16)
        beta_np.append(row)

    in_b = [
        {"aT": aT_np[c], "bT": bT_np[c // 4], "beta": beta_np[c // 4],
         "alpha": alpha_np[c]} for c in range(8)
    ]
    res_b = _run(ncb, in_b)

    # ---- assemble outputs
    conf = np.empty((N_BATCH, L, S), np.float32)
    maxs = np.empty((N_BATCH, L), np.float32)
    for c in range(8):
        n, l0 = c // 4, (c % 4) * R
        conf[n, l0:l0 + R, :] = res_b[c]["conf"]
        maxs[n, l0:l0 + R] = res_b[c]["rowmax"].T.reshape(R)

    mask_v = np.zeros((N_BATCH, L), bool)
    all_j_ids = np.zeros((N_BATCH, L), np.int32)
    mconf = np.zeros((N_BATCH, L), np.float32)
    cand = maxs > THR  # exact: device rowmax is the bitwise max of returned conf
    for n in range(N_BATCH):
        idx = np.nonzero(cand[n])[0]
        if idx.size == 0:
            continue
        colmax = conf[n].max(axis=0)  # [S]
        for l in idx:
            row = conf[n, l]
            m = (row > THR) & (row == row.max()) & (row == colmax)
            if m.any():
                j = int(np.argmax(m))
                mask_v[n, l] = True
                all_j_ids[n, l] = j
                mconf[n, l] = row[j]
    num_matches = mask_v.sum(axis=1).astype(np.int32)
    return conf, mask_v, all_j_ids, mconf, num_matches


def _kernel_numpy(feat_c0, feat_c1, mask_c0, mask_c1):
    """General-mask fallback (not used for the spec's all-ones masks)."""
    INF = 1e9
    conf = np.empty((N_BATCH, L, S), np.float32)
    for n in range(N_BATCH):
        sim = (feat_c0[n] / 16.0) @ (feat_c1[n] / 16.0).T / TEMP
        valid = mask_c0[n][:, None] & mask_c1[n][None, :]
        sim = np.where(valid, sim, -INF).astype(np.float32)
        e1 = np.exp(sim - sim.max(axis=0, keepdims=True))
        s1 = e1 / e1.sum(axis=0, keepdims=True)
        e2 = np.exp(sim - sim.max(axis=1, keepdims=True))
        s2 = e2 / e2.sum(axis=1, keepdims=True)
        conf[n] = s1 * s2
    max_s = conf.max(axis=2, keepdims=True)
    max_l = conf.max(axis=1, keepdims=True)
    mask = (conf > THR) & (conf == max_s) & (conf == max_l)
    mask_v = mask.any(axis=2)
    all_j_ids = np.argmax(mask, axis=2).astype(np.int32)
    mconf = np.take_along_axis(conf, all_j_ids[..., None], axis=2)[..., 0]
    mconf = mconf * mask_v.astype(np.float32)
    num_matches = mask_v.sum(axis=1).astype(np.int32)
    return conf, mask_v, all_j_ids, mconf, num_matches
